# revision 12
# baseline (speedup 1.0000x reference)
"""FourierKAN adapter kernel for Trainium2 (8 NeuronCores, SPMD data-parallel).

out[t, d] = x[t, d] + c0[d] + sum_{k=1..3} a_k[d] sin(k x) + b_k[d] cos(k x)
x: [32768, 1024] f32, coeffs: [1024, 7] f32.

Math (phase form): a sin(kx) + b cos(kx) = r_k sin(k x + phi_k).
Per harmonic, with w = f16(x / 2pi) and PHI_k[d] = phi_k / 2pi:
    t   = k*w + PHI_k                       (fp32, inside a fused DVE op)
    n   = (t + 2^23*1.5) - 2^23*1.5         (fp32 magic round-to-int)
    u_k = t - n                             (|u_k| <= 0.5, f16 out)
    s_k = Sin(2pi * u_k)                    (ScalarE spline, in-domain)
    m_k = s_k * r_k                         (DVE)
The whole t/n/u chain is ONE custom fused DVE instruction (FOURIER_RED_ANT)
per harmonic. The three Sins run as a single batched ScalarE activation over
a [128, 3*F] tile. PSUM accumulates m1+m2+m3 (f16 identity matmuls)
+ c0 (ones-row matmul) + x (float32r identity matmul, full PE rate);
ScalarE evacuates PSUM -> SBUF f32.

Sharding: x row-sharded across 8 cores; tables replicated.
"""

import math
import os

import numpy as np

T = 32768
D = 1024
K = 3
N_CORES = 8
T_CORE = T // N_CORES  # 4096
P = 128
F = 2048               # megatile free dim (= 2 d-periods)
M32 = 12582912.0       # 1.5 * 2^23: fp32 round-to-nearest-int magic constant
TWO_PI = 2.0 * math.pi

LAST_RESULTS = None
_CACHED = {}


def _register_fred():
    """Register the fused range-reduction custom DVE op (idempotent).

    FOURIER_RED_ANT: out = t - ((t + C0) - C0) with t = Src0*C1 + Src1.
    C0 = M32 makes the inner add/sub a round-to-nearest-integer, so
    out = frac(t) in [-0.5, 0.5]. 5 ALU stages, fp32 internal, f16 out.
    """
    import concourse.dve_ops as dve_ops
    from concourse.dve_spec import C0, C1, Spec, Src0, Src1, lower, _has_src1
    from concourse.dve_uop import DveOpSpec

    name = "FOURIER_RED_ANT"
    for op in dve_ops.OPS:
        if op.name == name:
            return op

    t = Src0 * C1 + Src1
    n = (t + C0) - C0
    spec = Spec(
        body=t - n,
        reference=lambda in0, in1, s0, s1, imm2: (
            lambda tt: tt - (np.float32(tt + np.float32(s0)) - np.float32(s0))
        )(np.float32(in0) * np.float32(s1) + np.float32(in1)),
    )

    row = dve_ops._CUSTOM_DVE_ROW_BASE + len(dve_ops.OPS)
    assert row < 0x20, "custom DVE opcode rows exhausted"
    dve_ops._SUB_OPCODE_FOR_NAME[name] = row

    shas = {}
    for ver in ("v3", "v4"):
        s = DveOpSpec(
            name=name, opcode=row, uops=lower(spec, ver=ver),
            rd1_en=_has_src1(spec),
        )
        shas[ver] = s.sha(ver)
    op = dve_ops.DveOp(name, spec, False, shas)
    dve_ops.OPS.append(op)
    return op


def _build_nc(mode="full"):
    from concourse import bacc
    import concourse.mybir as mybir
    from concourse import tile

    f32 = mybir.dt.float32
    f32r = mybir.dt.float32r
    f16 = mybir.dt.float16
    Sin = mybir.ActivationFunctionType.Sin
    Copy = mybir.ActivationFunctionType.Copy

    fred = _register_fred()

    nc = bacc.Bacc("TRN2", target_bir_lowering=False, debug=False)

    x = nc.dram_tensor("x", [T_CORE, D], f32, kind="ExternalInput").ap()
    out = nc.dram_tensor("out", [T_CORE, D], f32, kind="ExternalOutput").ap()

    phi = {}
    for k in (1, 2, 3):
        phi[k] = nc.dram_tensor(f"phi{k}", [P, F], f16, kind="ExternalInput").ap()
    rall = nc.dram_tensor("rall", [P, 3 * F], f16, kind="ExternalInput").ap()
    c0row = nc.dram_tensor("c0row", [1, F], f16, kind="ExternalInput").ap()
    id16 = nc.dram_tensor("id16", [P, P], f16, kind="ExternalInput").ap()
    ones1 = nc.dram_tensor("ones1", [1, P], f16, kind="ExternalInput").ap()

    xv = x.rearrange("(a b) d -> a (b d)", b=F // D)     # [2048, 2048]
    ov = out.rearrange("(a b) d -> a (b d)", b=F // D)
    n_tiles = xv.shape[0] // P  # 16

    use_gp = os.environ.get("KV2_GP", "1") == "1"

    with tile.TileContext(nc) as tc:
        with (
            tc.tile_pool(name="consts", bufs=1) as cpool,
            tc.tile_pool(name="io", bufs=int(os.environ.get("KV2_IOBUFS", "5"))) as iopool,
            tc.tile_pool(name="work", bufs=int(os.environ.get("KV2_WBUFS", "7"))) as pool,
            tc.tile_pool(name="x16p", bufs=3) as xpool,
            tc.tile_pool(name="psum", bufs=2, space="PSUM") as ppool,
        ):
            phit = {}
            for k in (1, 2, 3):
                t_ = cpool.tile([P, F], f16, tag=f"phi{k}")
                nc.sync.dma_start(out=t_[:], in_=phi[k])
                phit[k] = t_
            rallt = cpool.tile([P, 3 * F], f16, tag="rall")
            nc.sync.dma_start(out=rallt[:], in_=rall)
            c0t = cpool.tile([1, F], f16, tag="c0row")
            nc.sync.dma_start(out=c0t[:], in_=c0row)
            id16t = cpool.tile([P, P], f16, tag="id16")
            nc.sync.dma_start(out=id16t[:], in_=id16)
            ones1t = cpool.tile([1, P], f16, tag="ones1")
            nc.sync.dma_start(out=ones1t[:], in_=ones1)

            for i in range(n_tiles):
                xt = iopool.tile([P, F], f32, tag="xt")
                nc.sync.dma_start(out=xt[:], in_=xv[i * P:(i + 1) * P])

                if mode == "dma":
                    nc.sync.dma_start(out=ov[i * P:(i + 1) * P], in_=xt[:])
                    continue

                x16 = xpool.tile([P, F], f16, tag="x16")
                nc.scalar.activation(x16[:], xt[:], Copy, bias=0.0, scale=1.0)

                u = pool.tile([P, 3 * F], f16, tag="u")
                for k in (1, 2, 3):
                    nc.vector._custom_dve(
                        fred,
                        out=u[:, (k - 1) * F:k * F],
                        in0=xt[:], in1=phit[k][:],
                        s0=M32, s1=float(k / TWO_PI),
                    )

                # Sin in-place on u, then amplitude multiply in-place:
                # m (== u tile) = sin(2*pi*u) * r
                s = u
                nc.scalar.activation(s[:], u[:], Sin, bias=0.0,
                                     scale=float(TWO_PI))

                m = s
                nc.vector.tensor_mul(out=m[:], in0=s[:], in1=rallt[:])

                # m-matmuls first (psum allocated only once m is ready),
                # then x16 + c0 accumulate, per half-tile psum for finer
                # rotation granularity.
                nchunk = F // 512
                halves = 2
                hw_ = F // halves  # 1024
                hch = hw_ // 512
                ot = iopool.tile([P, F], f32, tag="ot")
                for h in range(halves):
                    ps = ppool.tile([P, hw_], f32, tag=f"ps{h}")
                    for c in range(hch):
                        sl_ps = slice(c * 512, (c + 1) * 512)
                        base = h * hw_ + c * 512
                        for ki in range(3):
                            slm = slice(ki * F + base, ki * F + base + 512)
                            nc.tensor.matmul(ps[:, sl_ps], id16t[:], m[:, slm],
                                             start=(ki == 0), stop=False)
                        nc.tensor.matmul(ps[:, sl_ps], id16t[:],
                                         x16[:, base:base + 512],
                                         start=False, stop=False)
                        nc.tensor.matmul(ps[:, sl_ps], ones1t[:],
                                         c0t[:, base:base + 512],
                                         start=False, stop=True)
                    nc.scalar.activation(ot[:, h * hw_:(h + 1) * hw_], ps[:],
                                         Copy, bias=0.0, scale=1.0)
                nc.sync.dma_start(out=ov[i * P:(i + 1) * P], in_=ot[:])

    nc.compile()
    return nc


def _host_tables(coeffs: np.ndarray) -> dict:
    c = coeffs.astype(np.float64)
    nrep = F // D
    tabs = {"c0row": np.tile(np.float16(c[:, 0]), nrep)[None, :]}
    rparts = []
    for k in (1, 2, 3):
        a = c[:, 2 * k - 1]
        b = c[:, 2 * k]
        r = np.hypot(a, b)
        phi = np.arctan2(b, a)
        tabs[f"phi{k}"] = np.tile(
            np.float16(phi / TWO_PI), (P, nrep))
        rparts.append(np.tile(np.float16(r), nrep))
    tabs["rall"] = np.tile(np.concatenate(rparts)[None, :], (P, 1))
    tabs["id16"] = np.eye(P, dtype=np.float16)
    tabs["ones1"] = np.ones((1, P), dtype=np.float16)
    return tabs


def kernel(x: np.ndarray, coeffs: np.ndarray) -> np.ndarray:
    global LAST_RESULTS
    from concourse.bass_utils import run_bass_kernel_spmd

    x = np.ascontiguousarray(np.asarray(x, dtype=np.float32))
    coeffs = np.asarray(coeffs, dtype=np.float32)
    assert x.shape == (T, D) and coeffs.shape == (D, 2 * K + 1)

    mode = os.environ.get("KV2_MODE", "full")
    key = ("nc", mode)
    if key not in _CACHED:
        _CACHED[key] = _build_nc(mode)
    nc = _CACHED[key]

    tabs = _host_tables(coeffs)
    in_maps = []
    for i in range(N_CORES):
        m = {"x": x[i * T_CORE:(i + 1) * T_CORE]}
        m.update(tabs)
        in_maps.append(m)

    trace = bool(os.environ.get("BASS_TRACE"))
    try:
        res = run_bass_kernel_spmd(
            nc, in_maps, list(range(N_CORES)), trace=trace,
        )
    except ModuleNotFoundError:
        res = run_bass_kernel_spmd(
            nc, in_maps, list(range(N_CORES)), trace=False,
        )
    LAST_RESULTS = res
    out = np.concatenate([res.results[i]["out"] for i in range(N_CORES)], axis=0)
    return out.astype(np.float32)
